# revision 12
# baseline (speedup 1.0000x reference)
"""PEER / product-key MoE routing kernel for Trainium2 (8 NeuronCores).

Strategy: data-parallel over tokens. Each of the 8 cores gets 256 of the
2048 tokens plus a full replica of the expert tables in its DRAM. Routing
(q projection, product-key scores, two-stage top-8), expert-row gathers,
and the PEER combine all run on-device. No collectives are needed; the
host only slices/packs inputs and concatenates the per-core outputs.

Key difference vs the first version: gathers are BATCHED through the
SWDGE `dma_gather` instruction (2048 rows per call) instead of 128
single-row indirect DMAs. Descriptor generation on GpSimd costs ~994 ns
fixed + 0.34 ns/descriptor per call, so the old per-slot scheme burned
~144 us of GpSimd time on fixed overhead alone, starving the DMA rings.
Two wrinkles make dma_gather non-trivial here:
  * its indices are signed int16, but expert ids span [0, 65536). The
    table base pointer is advanced by 32768 rows and ids are biased by
    -32768 on device; the SWDGE ucode sign-extends, so biased negative
    indices address the lower half of the table correctly (verified on
    HW by probe).
  * indices must be "wrapped": index i of a call lives at
    idxs[i % 16, i // 16], replicated across the 8 16-partition stripes.
    Routing produces ids as [token-partition, slot]; the 8:1 partition
    fan-in into the wrapped layout is done with 8 one-hot fp32 matmuls
    (lhsT slices of a host-built constant) plus strided PSUM->SBUF
    copies that also apply the -32768 bias and the int16 cast.

Per-core pipeline (per 128-token block, 64 expert slots):
  PE:  qT = Wq^T @ x^T (fp32, exact), s1/s2 = qT_half^T @ keysT (fp32),
       wrapped-index build (fp32 one-hot matmuls)
  DVE: top-8 of each 256-score set via max8/max_index (exact), top-8 of
       the 8x8 combo sums, winners' sub-key ids via is_equal one-hot
       reduction; softmax weights; inner products via fused
       tensor_tensor_reduce (page_down * x_bf16, one slot per op);
       va = relu(inner) * softmax weight (scalar_tensor_tensor).
  GpSimd: 4 dma_gather calls of 16 slots x 128 tokens each
       ([128, 16, 1024] bf16 pages; w_down/w_up packed side by side on
       the host so one 2 KB descriptor fetches both rows).
  ACT: in-place per-partition scale of each page's up-half by va.
  PE:  combine as PSUM-accumulated id @ scaled_up matmuls (a plain sum
       over slots; the identity lhsT never changes).

Routing is computed entirely in fp32, so expert selection matches the
fp32 reference exactly; only the expert tables are bf16 (rel err ~4e-3).
"""

import numpy as np

import concourse.bass as bass
import concourse.mybir as mybir
from concourse import bacc
from concourse.bass import IndirectOffsetOnAxis
from concourse.tile import TileContext
from concourse.bass_utils import run_bass_kernel_spmd

N_CORES = 8
N_HEADS = 8
D_KEYS = 128
HALF = 64
N_KEYS = 256
TOP_K = 8
D = 512
B = 2048           # total tokens
BC = B // N_CORES  # tokens per core (256)
TB = BC // 128     # token blocks per core (2)
CH = 8             # slots gathered per dma_gather call (8 chunks of 8 = 64)
F32 = mybir.dt.float32
U16 = mybir.dt.uint16
U32 = mybir.dt.uint32
I16 = mybir.dt.int16
I32 = mybir.dt.int32
BF16 = mybir.dt.bfloat16
X = mybir.AxisListType.X
OP = mybir.AluOpType
AF = mybir.ActivationFunctionType


def build_nc(stage="full", gather="dg"):
    nc = bacc.Bacc("TRN2", target_bir_lowering=False)

    xt_d = nc.dram_tensor("xt", [D, BC], F32, kind="ExternalInput")
    xb_d = nc.dram_tensor("xb", [BC, D], BF16, kind="ExternalInput")
    wq_d = nc.dram_tensor("wq", [D, N_HEADS * D_KEYS], F32, kind="ExternalInput")
    bqp_d = nc.dram_tensor("bqp", [HALF, 16], F32, kind="ExternalInput")
    kp1_d = nc.dram_tensor("kp1", [HALF, N_HEADS, N_KEYS], F32, kind="ExternalInput")
    kp2_d = nc.dram_tensor("kp2", [HALF, N_HEADS, N_KEYS], F32, kind="ExternalInput")
    wb_d = nc.dram_tensor("wb", [N_KEYS * N_KEYS, 2 * D], BF16,
                          kind="ExternalInput")
    id01_d = nc.dram_tensor("id01", [128, 128], BF16, kind="ExternalInput")
    wrapc_d = nc.dram_tensor("wrapc", [128, 8, 128], F32, kind="ExternalInput")
    out_d = nc.dram_tensor("out", [BC, D], F32, kind="ExternalOutput")

    with TileContext(nc) as tc:
        with (
            tc.tile_pool(name="const", bufs=1) as cpool,
            tc.tile_pool(name="qt", bufs=1) as qtpool,
            tc.tile_pool(name="psq", bufs=2, space="PSUM") as psq,
            tc.tile_pool(name="pss", bufs=2, space="PSUM") as pss,
            tc.tile_pool(name="sc", bufs=4) as scpool,
            tc.tile_pool(name="st2", bufs=1) as st2,
            tc.tile_pool(name="eqs", bufs=2) as eqs,
            tc.tile_pool(name="pgp", bufs=3) as pgp,
            tc.tile_pool(name="pacc", bufs=1, space="PSUM") as paccp,
            tc.tile_pool(name="accp", bufs=2) as accp,
        ):
            # ---- constant loads ----
            wq_sb = []
            xt_sb = []
            for k in range(4):
                t = cpool.tile([128, N_HEADS * D_KEYS], F32, tag=f"wq{k}")
                nc.sync.dma_start(out=t[:], in_=wq_d[k * 128:(k + 1) * 128, :])
                wq_sb.append(t)
                t2 = cpool.tile([128, BC], F32, tag=f"xt{k}")
                nc.sync.dma_start(out=t2[:], in_=xt_d[k * 128:(k + 1) * 128, :])
                xt_sb.append(t2)
            xb_sb = []
            for tb in range(TB):
                t = cpool.tile([128, D], BF16, tag=f"xb{tb}")
                nc.sync.dma_start(out=t[:], in_=xb_d[tb * 128:(tb + 1) * 128, :])
                xb_sb.append(t)
            kp1_sb = cpool.tile([HALF, N_HEADS, N_KEYS], F32, tag="kp1")
            nc.sync.dma_start(out=kp1_sb[:], in_=kp1_d[:, :, :])
            kp2_sb = cpool.tile([HALF, N_HEADS, N_KEYS], F32, tag="kp2")
            nc.sync.dma_start(out=kp2_sb[:], in_=kp2_d[:, :, :])
            id01_sb = cpool.tile([128, 128], BF16, tag="id01")
            nc.sync.dma_start(out=id01_sb[:], in_=id01_d[:, :])
            bqp_sb = cpool.tile([HALF, 16], F32, tag="bqp")
            nc.sync.dma_start(out=bqp_sb[:], in_=bqp_d[:, :])
            wrapc_sb = cpool.tile([128, 8, 128], F32, tag="wrapc")
            nc.sync.dma_start(out=wrapc_sb[:], in_=wrapc_d[:, :, :])
            iota8 = cpool.tile([128, 8], U16, tag="iota8")
            nc.gpsimd.iota(iota8[:], pattern=[[1, 8]], base=0, channel_multiplier=0)

            # ---- routing, per token block ----
            # qT: [feature, token], 16 column-tiles of 64 features.
            # feature f = m*128 + j*64 + p  ->  column mj = 2*m + j
            qt_all = qtpool.tile([HALF, 16, BC], F32, tag="qt_all")
            idx16s = {}
            c16s = {}
            idx32 = {}
            ws = {}
            v8s = {}
            for tb in range(TB):
                tsl = slice(tb * 128, (tb + 1) * 128)
                for mj in range(16):
                    ps = psq.tile([HALF, 128], F32, tag="psq")
                    for k in range(4):
                        nc.tensor.matmul(
                            out=ps[:],
                            lhsT=wq_sb[k][:, mj * HALF:(mj + 1) * HALF],
                            rhs=xt_sb[k][:, tsl],
                            start=(k == 0),
                            stop=(k == 3),
                        )
                    # add per-feature bias (per-partition scalar), to SBUF
                    nc.vector.tensor_scalar(
                        out=qt_all[:, mj, tsl], in0=ps[:],
                        scalar1=bqp_sb[:, mj:mj + 1], scalar2=None, op0=OP.add,
                    )

                s1t = st2.tile([128, 64], F32, tag=f"s1t{tb}")
                s2t = st2.tile([128, 64], F32, tag=f"s2t{tb}")
                i1 = st2.tile([128, 64], U16, tag=f"i1{tb}")
                i2 = st2.tile([128, 64], U16, tag=f"i2{tb}")
                # ---- scores + stage-1 top8 (exact) ----
                for m in range(N_HEADS):
                    for half, (kp, st_, ix) in enumerate(
                        ((kp1_sb, s1t, i1), (kp2_sb, s2t, i2))
                    ):
                        ps = pss.tile([128, N_KEYS], F32, tag="pss")
                        nc.tensor.matmul(
                            out=ps[:],
                            lhsT=qt_all[:, 2 * m + half, tsl],
                            rhs=kp[:, m, :],
                            start=True, stop=True,
                        )
                        s_sb = scpool.tile([128, N_KEYS], F32, tag="s_sb")
                        nc.scalar.copy(out=s_sb[:], in_=ps[:])
                        nc.vector.max(out=st_[:, m * 8:(m + 1) * 8], in_=s_sb[:])
                        nc.vector.max_index(
                            out=ix[:, m * 8:(m + 1) * 8],
                            in_max=st_[:, m * 8:(m + 1) * 8],
                            in_values=s_sb[:],
                        )

                # ---- stage-2: 8x8 combo scores, top8 ----
                cs = st2.tile([128, 512], F32, tag=f"cs{tb}")
                for m in range(N_HEADS):
                    nc.vector.tensor_tensor(
                        out=cs[:, m * 64:(m + 1) * 64].rearrange(
                            "p (a b) -> p a b", a=8),
                        in0=s1t[:, m * 8:(m + 1) * 8].unsqueeze(2).to_broadcast(
                            [128, 8, 8]),
                        in1=s2t[:, m * 8:(m + 1) * 8].unsqueeze(1).to_broadcast(
                            [128, 8, 8]),
                        op=OP.add,
                    )
                v8 = st2.tile([128, 64], F32, tag=f"v8{tb}")
                n8 = st2.tile([128, 64], U16, tag=f"n8{tb}")
                for m in range(N_HEADS):
                    nc.vector.max(out=v8[:, m * 8:(m + 1) * 8],
                                  in_=cs[:, m * 64:(m + 1) * 64])
                    nc.vector.max_index(
                        out=n8[:, m * 8:(m + 1) * 8],
                        in_max=v8[:, m * 8:(m + 1) * 8],
                        in_values=cs[:, m * 64:(m + 1) * 64])
                k1 = st2.tile([128, 64], U16, tag=f"k1{tb}")
                nc.vector.tensor_scalar(
                    out=k1[:], in0=n8[:], scalar1=3, scalar2=None,
                    op0=OP.logical_shift_right)
                k2 = st2.tile([128, 64], U16, tag=f"k2{tb}")
                nc.vector.tensor_scalar(
                    out=k2[:], in0=n8[:], scalar1=7, scalar2=None,
                    op0=OP.bitwise_and)

                # resolve winners' sub-key ids: isel[p,m,j] = i[p,m,k1[p,m,j]]
                sels = []
                for kk, ix in ((k1, i1), (k2, i2)):
                    eq = eqs.tile([128, 512], U16, tag="eq")
                    nc.vector.tensor_tensor(
                        out=eq[:, :].rearrange("p (m j k) -> p m j k", m=8, j=8),
                        in0=kk[:, :].rearrange("p (m j) -> p m j", m=8)
                            .unsqueeze(3).to_broadcast([128, 8, 8, 8]),
                        in1=iota8[:, :].unsqueeze(1).unsqueeze(1)
                            .to_broadcast([128, 8, 8, 8]),
                        op=OP.is_equal)
                    prod = eqs.tile([128, 512], U16, tag="prod")
                    nc.vector.tensor_tensor(
                        out=prod[:, :].rearrange("p (m j k) -> p m j k", m=8, j=8),
                        in0=eq[:, :].rearrange("p (m j k) -> p m j k", m=8, j=8),
                        in1=ix[:, :].rearrange("p (m k) -> p m k", m=8)
                            .unsqueeze(2).to_broadcast([128, 8, 8, 8]),
                        op=OP.mult)
                    sel = st2.tile([128, 64], U16, tag=f"sel{len(sels)}{tb}")
                    with nc.allow_low_precision(
                            reason="one-hot uint16 sum, values <= 255"):
                        nc.vector.reduce_sum(
                            out=sel[:],
                            in_=prod[:, :].rearrange("p (mj k) -> p mj k", k=8),
                            axis=X)
                    sels.append(sel)
                idx16 = st2.tile([128, 64], U16, tag=f"idx16{tb}")
                nc.vector.tensor_scalar(
                    out=idx16[:], in0=sels[0][:], scalar1=256, scalar2=None,
                    op0=OP.mult)
                nc.vector.tensor_tensor(
                    out=idx16[:], in0=idx16[:], in1=sels[1][:], op=OP.add)
                idx16s[tb] = idx16
                v8s[tb] = v8

                if gather == "dg":
                    # wrapped-index build: c16[p, 8s+h] = idx[16h + p%16, s]
                    # - 32768, replicated across the 8 16-partition stripes.
                    idxf = st2.tile([128, 64], F32, tag=f"idxf{tb}")
                    nc.vector.tensor_copy(out=idxf[:], in_=idx16[:])
                    c16 = st2.tile([128, 8 * 64], I16, tag=f"c16{tb}")
                    c16r = c16[:].rearrange("p (s h) -> p s h", h=8)
                    for h in range(8):
                        psw = psq.tile([128, 64], F32, tag="psq")
                        nc.tensor.matmul(
                            out=psw[:], lhsT=wrapc_sb[:, h, :], rhs=idxf[:],
                            start=True, stop=True)
                        nc.vector.tensor_scalar(
                            out=c16r[:, :, h], in0=psw[:],
                            scalar1=-32768.0, scalar2=None, op0=OP.add)
                    c16s[tb] = c16
                ix32 = st2.tile([128, 64], I32, tag=f"idx32{tb}")
                nc.vector.tensor_copy(out=ix32[:], in_=idx16[:])
                idx32[tb] = ix32

                # ---- softmax over each head's top-8 ----
                rmax = st2.tile([128, 8], F32, tag=f"rmax{tb}")
                nc.vector.reduce_max(
                    out=rmax[:], in_=v8[:, :].rearrange("p (m k) -> p m k", m=8),
                    axis=X)
                ex = st2.tile([128, 64], F32, tag=f"ex{tb}")
                nc.vector.tensor_tensor(
                    out=ex[:, :].rearrange("p (m k) -> p m k", m=8),
                    in0=v8[:, :].rearrange("p (m k) -> p m k", m=8),
                    in1=rmax[:, :].unsqueeze(2).to_broadcast([128, 8, 8]),
                    op=OP.subtract)
                nc.scalar.activation(out=ex[:], in_=ex[:], func=AF.Exp)
                rsum = st2.tile([128, 8], F32, tag=f"rsum{tb}")
                nc.vector.reduce_sum(
                    out=rsum[:], in_=ex[:, :].rearrange("p (m k) -> p m k", m=8),
                    axis=X)
                rinv = st2.tile([128, 8], F32, tag=f"rinv{tb}")
                nc.vector.reciprocal(out=rinv[:], in_=rsum[:])
                w8 = st2.tile([128, 64], F32, tag=f"w8{tb}")
                nc.vector.tensor_tensor(
                    out=w8[:, :].rearrange("p (m k) -> p m k", m=8),
                    in0=ex[:, :].rearrange("p (m k) -> p m k", m=8),
                    in1=rinv[:, :].unsqueeze(2).to_broadcast([128, 8, 8]),
                    op=OP.mult)
                ws[tb] = w8

            if stage == "routing":
                for tb in range(TB):
                    dbg = st2.tile([128, 64], F32, tag=f"dbg{tb}")
                    nc.vector.tensor_copy(out=dbg[:], in_=idx16s[tb][:])
                    nc.sync.dma_start(
                        out=out_d[tb * 128:(tb + 1) * 128, 0:64], in_=dbg[:])
                    nc.sync.dma_start(
                        out=out_d[tb * 128:(tb + 1) * 128, 64:128],
                        in_=v8s[tb][:])
            if stage == "wrapidx":
                for tb in range(TB):
                    dbg = st2.tile([128, 512], F32, tag=f"dbgw{tb}")
                    nc.vector.tensor_copy(out=dbg[:], in_=c16s[tb][:])
                    nc.sync.dma_start(
                        out=out_d[tb * 128:(tb + 1) * 128, 0:512], in_=dbg[:])

            # ---- main loop: batched gather, inner, scale, combine ----
            # dma_gather's completion semaphore fires before all rings have
            # drained (HW-observed), so chunk k-1 is only consumed after
            # chunk k's semaphore ("lag-one"): by ring-FIFO, one further
            # call's descriptors being fetched implies the previous call's
            # data descriptors have executed. A trailing dummy gather
            # protects the final chunk the same way.
            tbs = () if stage in ("routing", "wrapidx") else tuple(range(TB))
            NCH = 64 // CH
            inner = {}
            va = {}
            pacc = {}
            for tb in tbs:
                inner[tb] = st2.tile([128, 64], F32, tag=f"inner{tb}",
                                     name=f"inner{tb}")
                va[tb] = st2.tile([128, 64], F32, tag=f"va{tb}", name=f"va{tb}")
                pacc[tb] = paccp.tile([128, D], F32, tag=f"pacc{tb}",
                                      name=f"pacc{tb}")

            scr = st2.tile([128, D], BF16, tag="scr", name="scr") \
                if stage not in ("routing", "wrapidx") else None

            def consume(tb, c, page):
                csl = slice(c * CH, (c + 1) * CH)
                # inner products: fused multiply + free-dim accumulate
                # (TensorScalarPtr's accumulator; InstTensorTensorReduce
                # does not execute on this hardware build)
                for s in range(CH):
                    col = c * CH + s
                    nc.vector.scalar_tensor_tensor(
                        out=scr[:],
                        in0=page[:, s, 0:D],
                        scalar=1.0,
                        in1=xb_sb[tb][:],
                        op0=OP.mult, op1=OP.mult,
                        accum_out=inner[tb][:, col:col + 1],
                    )
                if stage == "inner":
                    return
                # va = relu(inner) * softmax_weight
                if stage == "wdown":
                    nc.vector.tensor_scalar(
                        out=va[tb][:, csl], in0=inner[tb][:, csl],
                        scalar1=0.0, scalar2=None, op0=OP.max)
                else:
                    nc.vector.scalar_tensor_tensor(
                        out=va[tb][:, csl], in0=inner[tb][:, csl],
                        scalar=0.0, in1=ws[tb][:, csl],
                        op0=OP.max, op1=OP.mult)
                # scale the up-half rows in place, then sum on PE
                for s in range(CH):
                    col = c * CH + s
                    nc.scalar.activation(
                        out=page[:, s, D:2 * D], in_=page[:, s, D:2 * D],
                        func=AF.Copy, scale=va[tb][:, col:col + 1])
                    nc.tensor.matmul(
                        out=pacc[tb][:], lhsT=id01_sb[:],
                        rhs=page[:, s, D:2 * D],
                        start=(col == 0), stop=(col == 63))

            def finish(tb):
                if stage == "gather":
                    return
                if stage == "inner":
                    nc.sync.dma_start(
                        out=out_d[tb * 128:(tb + 1) * 128, 0:64],
                        in_=inner[tb][:])
                    return
                acc_sb = accp.tile([128, D], F32, tag=f"acc{tb}")
                nc.scalar.copy(out=acc_sb[:], in_=pacc[tb][:])
                if stage == "wdown":
                    nc.sync.dma_start(
                        out=out_d[tb * 128:(tb + 1) * 128, 0:64],
                        in_=inner[tb][:])
                else:
                    nc.sync.dma_start(
                        out=out_d[tb * 128:(tb + 1) * 128, :], in_=acc_sb[:])

            chunks = [(tb, c) for tb in tbs for c in range(NCH)]
            if gather == "dg" and chunks:
                touch = st2.tile([128, 2], F32, tag="touch", name="touch")
                pages = {}
                prev = None
                for tb, c in chunks:
                    page = pgp.tile([128, CH, 2 * D], BF16, tag="pg",
                                    name=f"pg{tb}_{c}")
                    pages[(tb, c)] = page
                    nc.gpsimd.dma_gather(
                        out_ap=page[:],
                        in_ap=wb_d[N_KEYS * N_KEYS // 2:, :],
                        idxs_ap=c16s[tb][:, c * 8 * CH:(c + 1) * 8 * CH],
                        num_idxs=128 * CH,
                        num_idxs_reg=128 * CH,
                        elem_size=2 * D,
                    )
                    # re-gather the call's final column via the reliable
                    # indirect path: the dma_gather tail descriptor can
                    # still be in flight when its semaphore fires, and both
                    # writes carry identical bytes, so the overwrite order
                    # is irrelevant while consumers get a trustworthy
                    # completion semaphore for the tail cells.
                    nc.gpsimd.indirect_dma_start(
                        out=page[:, CH - 1, :], out_offset=None,
                        in_=wb_d[:, :],
                        in_offset=IndirectOffsetOnAxis(
                            ap=idx32[tb][:, c * CH + CH - 1:c * CH + CH],
                            axis=0),
                    )
                    # ordering-only read: consumption below starts no
                    # earlier than this chunk's gather semaphore
                    nc.vector.tensor_copy(out=touch[:, 0:1],
                                          in_=page[:, 0, 0:1])
                    if prev is not None:
                        consume(*prev, pages[prev])
                        if prev[0] != tb:
                            finish(prev[0])
                    prev = (tb, c)
                # trailing dummy gather shields the last real chunk
                pgd = pgp.tile([128, 2, 2 * D], BF16, tag="pgd", name="pgd")
                nc.gpsimd.dma_gather(
                    out_ap=pgd[:],
                    in_ap=wb_d[N_KEYS * N_KEYS // 2:, :],
                    idxs_ap=c16s[tbs[-1]][:, 0:16],
                    num_idxs=256,
                    num_idxs_reg=256,
                    elem_size=2 * D,
                )
                nc.vector.tensor_copy(out=touch[:, 1:2], in_=pgd[:, 0, 0:1])
                consume(*prev, pages[prev])
                finish(prev[0])
            else:
                for tb, c in chunks:
                    csl = slice(c * CH, (c + 1) * CH)
                    page = pgp.tile([128, CH, 2 * D], BF16, tag="pg",
                                    name=f"pg{tb}_{c}")
                    for s in range(CH):
                        col = c * CH + s
                        nc.gpsimd.indirect_dma_start(
                            out=page[:, s, :], out_offset=None,
                            in_=wb_d[:, :],
                            in_offset=IndirectOffsetOnAxis(
                                ap=idx32[tb][:, col:col + 1], axis=0),
                        )
                    if stage == "gather":
                        continue
                    consume(tb, c, page)
                    if c == NCH - 1:
                        finish(tb)

    nc.compile()
    return nc


_NC_CACHE = None


def _get_nc():
    global _NC_CACHE
    if _NC_CACHE is None:
        _NC_CACHE = build_nc()
    return _NC_CACHE


def _make_wrapc():
    wrapc = np.zeros((128, 8, 128), dtype=np.float32)
    for t in range(128):
        h, q = divmod(t, 16)
        wrapc[t, h, q::16] = 1.0
    return wrapc


def _prep_in_maps(inputs):
    import ml_dtypes
    q = np.ascontiguousarray(np.asarray(inputs["queries"], dtype=np.float32))
    Wq = np.ascontiguousarray(np.asarray(inputs["Wq"], dtype=np.float32))
    bq = np.asarray(inputs["bq"], dtype=np.float32)
    keys = np.asarray(inputs["keys"], dtype=np.float32)
    wd = np.asarray(inputs["w_down"], dtype=np.float32)
    wu = np.asarray(inputs["w_up"], dtype=np.float32)
    wb = np.ascontiguousarray(
        np.concatenate([wd, wu], axis=1).astype(ml_dtypes.bfloat16))
    id01 = np.eye(128, dtype=np.float32).astype(ml_dtypes.bfloat16)
    wrapc = _make_wrapc()

    x = q.reshape(B, D)
    # bqp[p, mj] = bq[mj*64 + p]
    bqp = np.ascontiguousarray(bq.reshape(16, HALF).T)
    # kp{1,2}[c, m, n] = keys[m, half, n, c]
    kp1 = np.ascontiguousarray(keys[:, 0].transpose(2, 0, 1))
    kp2 = np.ascontiguousarray(keys[:, 1].transpose(2, 0, 1))

    in_maps = []
    for c in range(N_CORES):
        xc = x[c * BC:(c + 1) * BC]
        in_maps.append({
            "xt": np.ascontiguousarray(xc.T),
            "xb": np.ascontiguousarray(xc.astype(ml_dtypes.bfloat16)),
            "wq": Wq,
            "bqp": bqp,
            "kp1": kp1,
            "kp2": kp2,
            "wb": wb,
            "id01": id01,
            "wrapc": wrapc,
        })
    return in_maps


def run(inputs, trace=False):
    """Run on 8 NeuronCores; returns (out [2,1024,512], BassKernelResults)."""
    nc = _get_nc()
    in_maps = _prep_in_maps(inputs)
    res = run_bass_kernel_spmd(
        nc, in_maps, core_ids=list(range(N_CORES)), trace=trace)
    out = np.concatenate(
        [res.results[c]["out"] for c in range(N_CORES)], axis=0)
    return out.reshape(2, 1024, D), res


def kernel(**inputs) -> np.ndarray:
    out, _ = run(inputs, trace=False)
    return out
